# revision 9
# baseline (speedup 1.0000x reference)
"""BinaryConv2d (3x3, stride 1, pad 1) on 8 TRN2 NeuronCores.

Data-parallel: batch 32 sharded 4-per-core; weight/bias replicated.

Algorithm: 1-D Winograd F(2,3) along H for rows 0..51 + direct conv for
rows 52..55. For each pair of output rows (tile row t) the conv needs 4
H-transformed input rows
  U0 = x[2t-1] - x[2t+1]   U1 = x[2t] + x[2t+1]
  U2 = x[2t+1] - x[2t]     U3 = x[2t] - x[2t+2]
and 4 transformed weight sets Wt[a][o,i,dw] = sum_dh G[a,dh] w[o,i,dh,dw]
(entries +-0.5/+-1.5/+-1, bf16-exact for binarized weights). Then
  V[a] = sum_dw Wt[a][:,:,dw] @ U[a] shifted by dw   (3 matmuls, PSUM)
  y[2t]   = V0 + V1 + V2 + bias
  y[2t+1] = V1 - V2 - V3 + bias
12 matmuls per 14 output rows per 128-channel half instead of the direct
conv's 18 -- 1.5x less PE work.

Why the direct-conv rows: measured engine occupancy shows PE/DVE/ACT all
~72-76us busy -- the kernel is 4-way balanced, not PE-bound. Winograd's
combine costs 4 DVE + 2 ACT PSUM-reading ops per group-half (one-PSUM-
operand-per-op limit) and that work CANNOT move to GpSimd (no PSUM
access). Direct-conv rows need ~zero vector work (PE accumulates all 9
taps in PSUM, one ACT bias-copy moves the tile out) at 1.5x the PE
cycles, so shifting 4 rows/image to direct trades ~5us of DVE+ACT for
~2us of PE -- converging the walls.

Width padding is avoided entirely: the dw=1 (center) tap runs first at
full width with start=True, then the dw=0/2 taps accumulate into
column-shifted PSUM windows -- the skipped edge column is exactly the
tap's zero-pad contribution.

Engine split:
 - PE: 12 MMs per (group, half) at N=392/336; 9 direct MMs per image-
   half at N=224; LDWEIGHTS hides under the MM spacing. Warmup dummy
   matmuls (gated only on a DVE memset, not GpSimd's ~6us preamble)
   bridge the fill window so the HAM clock-gate lifts early.
 - DVE: the 4 combine ops per (group, half) + image 0's U-transform
   pieces + warm_w memset.
 - GpSimd: U transforms for images 1..3 (staged two images ahead) +
   late-needed weight DMAs (half1, direct weights, bias).
 - ACT: bias folding (c0 = V0+b, c3 = b-V3, PSUM reads), direct-path
   bias copies (alternating with DVE), xs edge memsets.
 - Sync: half0 weight DMAs first (MM #1's gate, ~0.6us HWDGE issue),
   then image DMAs (contiguous 6272B/partition into the H-padded slab;
   image 0 in 4 row-chunks), all output DMAs.

Output is fp16 (halves the 12.8MB/core output DMA; host upcasts;
measured rel err 3.0e-3 vs the 2e-2 budget).
"""

import numpy as np
from contextlib import ExitStack

import concourse.bass as bass
import concourse.bacc as bacc
import concourse.mybir as mybir
import concourse.tile as tile
from concourse.bass_utils import run_bass_kernel_spmd

N_CORES = 8
N_BATCH = 32
N_PER_CORE = N_BATCH // N_CORES  # 4
C_IN = 128
C_OUT = 256
H = W = 56
HP = H + 2           # zero-padded height (in xs)
R_WINO = 52          # rows 0..51 via Winograd, 52..55 direct
T_IMG = R_WINO // 2  # 26 tile rows per image
GROUPS = ((0, 7), (7, 14), (14, 20), (20, 26))  # tile-row ranges
R_DIR = H - R_WINO   # 4 direct rows

f32 = mybir.dt.float32
bf16 = mybir.dt.bfloat16
f16 = mybir.dt.float16
fp8 = mybir.dt.float8e4
AF = mybir.ActivationFunctionType


def build_program() -> bass.Bass:
    nc = bacc.Bacc("TRN2", target_bir_lowering=False, debug=False)
    x = nc.dram_tensor("x", [N_PER_CORE, C_IN, H, W], bf16, kind="ExternalInput")
    # wt[half, i, a, dw, o]: host-transformed Winograd weights, fp8-exact.
    wt = nc.dram_tensor("wt", [2, C_IN, 4, 3, 128], fp8, kind="ExternalInput")
    # wd[i, half, dh, dw, o]: plain binarized weights for the direct rows.
    wd = nc.dram_tensor("wd", [C_IN, 2, 3, 3, 128], fp8, kind="ExternalInput")
    b = nc.dram_tensor("b", [C_OUT], f32, kind="ExternalInput")
    y = nc.dram_tensor("y", [N_PER_CORE, C_OUT, H, W], f16, kind="ExternalOutput")

    with tile.TileContext(nc) as tc, ExitStack() as ctx:
        singles = ctx.enter_context(tc.tile_pool(name="singles", bufs=1))
        xsp = ctx.enter_context(tc.tile_pool(name="xsp", bufs=4))
        up = ctx.enter_context(tc.tile_pool(name="up", bufs=3))
        psum_mm = ctx.enter_context(
            tc.tile_pool(name="psum_mm", bufs=8, space="PSUM")
        )
        tdp = ctx.enter_context(tc.tile_pool(name="tdp", bufs=8))
        obp = ctx.enter_context(tc.tile_pool(name="obp", bufs=4))

        # warmup weights come from the DVE (fast queue start); warmup MMs
        # then only gate on DVE, not GpSimd's ~6us preamble
        warm_w = singles.tile([128, 128], bf16)
        nc.vector.memset(warm_w, 0.0)

        wtile = singles.tile([128, 2, 4, 3, 128], fp8, name="wt")
        wdt = singles.tile([128, 2, 3, 3, 128], fp8, name="wd")
        bsb = singles.tile([128, 2], f32)

        def stage_image(n):
            """One contiguous DMA into the H-padded slab + edge memsets +
            the four H-transform ops (GpSimd; staged 2 images ahead)."""
            xs = xsp.tile([128, HP, W], bf16, name="xs")
            nc.sync.dma_start(out=xs[:, 1:1 + H, :], in_=x.ap()[n])
            nc.scalar.memzero(xs[:, 0, :])
            nc.scalar.memzero(xs[:, HP - 1, :])

            def e(k):  # rows k, k+2, ..., k+50 of the padded slab
                return xs[:, k:k + 2 * (T_IMG - 1) + 1:2, :]

            U = [up.tile([128, T_IMG, W], bf16, name=f"u{a}", tag=f"u{a}")
                 for a in range(4)]
            nc.gpsimd.tensor_sub(U[0], e(0), e(2))
            nc.gpsimd.tensor_add(U[1], e(1), e(2))
            nc.gpsimd.tensor_sub(U[2], e(2), e(1))
            nc.gpsimd.tensor_sub(U[3], e(1), e(3))
            return xs, U

        def stage_image0():
            """Image 0 rides the critical path: half0 weights issue from
            Sync FIRST (HWDGE ~0.6us issue; MM #1 gates on 196KB), then
            4 row-chunk image DMAs; late-needed weights (half1, direct,
            bias) issue from the GpSimd queue once its preamble ends."""
            xs = xsp.tile([128, HP, W], bf16, name="xs")
            nc.sync.dma_start(out=wtile[:, 0, 0], in_=wt.ap()[0][:, 0])
            nc.sync.dma_start(out=wtile[:, 0, 1:4], in_=wt.ap()[0][:, 1:4])
            nc.gpsimd.dma_start(out=wtile[:, 1], in_=wt.ap()[1])
            nc.gpsimd.dma_start(
                out=bsb, in_=b.ap().rearrange("(h o) -> o h", h=2))
            nc.gpsimd.dma_start(out=wdt, in_=wd.ap())
            chunks = [(0, 16), (16, 30), (30, 42), (42, 56)]
            for lo, hi in chunks:
                nc.sync.dma_start(out=xs[:, 1 + lo:1 + hi, :],
                                  in_=x.ap()[0, :, lo:hi, :])
            nc.scalar.memzero(xs[:, 0, :])
            nc.scalar.memzero(xs[:, HP - 1, :])

            U = [up.tile([128, T_IMG, W], bf16, name=f"u{a}", tag=f"u{a}")
                 for a in range(4)]

            def piece(t0, t1, eng):
                r = slice(t0, t1)

                def e(k):
                    lo = 2 * t0 + k
                    return xs[:, lo:lo + 2 * (t1 - t0 - 1) + 1:2, :]

                eng.tensor_sub(U[0][:, r, :], e(0), e(2))
                eng.tensor_add(U[1][:, r, :], e(1), e(2))
                eng.tensor_sub(U[2][:, r, :], e(2), e(1))
                eng.tensor_sub(U[3][:, r, :], e(1), e(3))

            # pieces 0-2 on DVE (fast, ahead of the combine backlog),
            # the last on GpSimd
            for (t0, t1) in GROUPS[:-1]:
                piece(t0, t1, nc.vector)
            piece(*GROUPS[-1], nc.gpsimd)
            return xs, U

        # ---- PE warmup: bridge the pipeline-fill window with dummy
        # matmuls so the HAM clock-gate (K=4/8 cold throttle) lifts
        # before the real stream starts.
        wp = psum_mm.tile([128, 128], f32, tag="ps")
        NWARM = 30
        for k in range(NWARM):
            nc.tensor.matmul(wp, lhsT=warm_w, rhs=warm_w,
                             start=(k == 0), stop=(k == NWARM - 1))

        def do_group(n, U, t0, t1, half, split_dma=False):
            """Output rows 2*t0..2*t1 of image n, one half."""
            tg = t1 - t0
            h0 = 2 * t0
            r = slice(t0, t1)
            nw = tg * W
            V = [None] * 4
            # last group only: a=3 first, so its bias-fold (c3) runs during
            # the remaining matmuls and only y0/y1 trail the last one
            for a in ((3, 0, 1, 2) if split_dma else (0, 1, 2, 3)):
                ps = psum_mm.tile([128, tg, W], f32, name=f"v{a}",
                                  tag="ps")
                lt = wtile[:, half, a]
                # center tap first at full width (sets has_written), then
                # the shifted taps accumulate into column-shifted windows
                nc.tensor.matmul(ps, lhsT=lt[:, 1], rhs=U[a][:, r, :],
                                 start=True, stop=False)
                nc.tensor.matmul(ps[:, :, 1:W], lhsT=lt[:, 0],
                                 rhs=U[a][:, r, 0:W - 1],
                                 start=False, stop=False)
                nc.tensor.matmul(ps[:, :, 0:W - 1], lhsT=lt[:, 2],
                                 rhs=U[a][:, r, 1:W],
                                 start=False, stop=True)
                V[a] = ps
            # y0 = V0+V1+V2+b, y1 = V1-V2-V3+b; ACT folds the bias into the
            # single-use terms, DVE does the two-tensor combines (max one
            # PSUM operand each).
            ob = obp.tile([128, tg, 2, W], f16, name="ob", tag="ob")
            c0 = tdp.tile([128, tg, W], f32, name="c0", tag="td")
            c3 = tdp.tile([128, tg, W], f32, name="c3", tag="td")
            t = tdp.tile([128, tg, W], f32, name="t", tag="td")
            e = tdp.tile([128, tg, W], f32, name="e", tag="td")

            def act_c0():
                nc.scalar.activation(c0, V[0], AF.Identity,
                                     bias=bsb[:, half:half + 1])

            def act_c3():
                nc.scalar.activation(c3, V[3], AF.Identity,
                                     bias=bsb[:, half:half + 1], scale=-1.0)

            ych = y.ap()[n, half * 128:(half + 1) * 128]
            if split_dma:
                # tail: c3/c0/t/e all complete during the a=0..2 matmuls
                # (a=3 ran first), so only y0/y1 trail the last matmul --
                # and they go in two row-blocks so the first block's DMA
                # overlaps the second block's compute.
                act_c3(), act_c0()
                nc.vector.tensor_add(t, c0, V[1])
                nc.vector.tensor_add(e, c3, V[1])
                for rlo, rhi in ((0, 3), (3, tg)):
                    nc.vector.tensor_add(ob[:, rlo:rhi, 0, :],
                                         t[:, rlo:rhi], V[2][:, rlo:rhi])
                    nc.vector.tensor_sub(ob[:, rlo:rhi, 1, :],
                                         e[:, rlo:rhi], V[2][:, rlo:rhi])
                    nc.sync.dma_start(
                        out=ych[:, h0 + 2 * rlo:h0 + 2 * rhi, :],
                        in_=ob[:, rlo:rhi],
                    )
            else:
                act_c0()
                nc.vector.tensor_add(t, c0, V[1])
                nc.vector.tensor_add(ob[:, :, 0, :], t, V[2])
                act_c3()
                nc.vector.tensor_add(e, c3, V[1])
                nc.vector.tensor_sub(ob[:, :, 1, :], e, V[2])
                nc.sync.dma_start(out=ych[:, h0:h0 + 2 * tg, :], in_=ob)

        def do_direct(n, xs, half):
            """Rows 52..55 of image n, one half: direct conv, all 9 taps
            accumulated in PSUM; one bias-copy (alternating ACT/DVE to
            balance load) moves the tile out."""
            ps = psum_mm.tile([128, R_DIR, W], f32, name="vd",
                              tag="ps")
            first = True
            for dh in range(3):
                # output rows 52..55 <- x rows 51+dh..54+dh = xs rows 52+dh..
                rx = xs[:, R_WINO + dh:R_WINO + dh + R_DIR, :]
                lt = wdt[:, half, dh]
                nc.tensor.matmul(ps, lhsT=lt[:, 1], rhs=rx,
                                 start=first, stop=False)
                first = False
                nc.tensor.matmul(ps[:, :, 1:W], lhsT=lt[:, 0],
                                 rhs=rx[:, :, 0:W - 1],
                                 start=False, stop=False)
                nc.tensor.matmul(ps[:, :, 0:W - 1], lhsT=lt[:, 2],
                                 rhs=rx[:, :, 1:W],
                                 start=False, stop=(dh == 2))
            ob = obp.tile([128, R_DIR, W], f16, name="obd", tag="obd")
            bias = bsb[:, half:half + 1]
            if (n + half) % 2 == 0:
                nc.scalar.activation(ob, ps, AF.Identity, bias=bias)
            else:
                nc.vector.tensor_scalar(out=ob, in0=ps, scalar1=1.0,
                                        scalar2=bias,
                                        op0=mybir.AluOpType.mult,
                                        op1=mybir.AluOpType.add)
            ych = y.ap()[n, half * 128:(half + 1) * 128]
            nc.sync.dma_start(out=ych[:, R_WINO:H, :], in_=ob)

        # software pipeline: staged two images ahead so GpSimd's slower
        # transform rate never gates the PE
        imgs = [stage_image0(), stage_image(1), stage_image(2)]
        for n in range(N_PER_CORE):
            if n + 3 < N_PER_CORE:
                imgs.append(stage_image(n + 3))
            xs, U = imgs[n]
            for gi, (t0, t1) in enumerate(GROUPS):
                for half in range(2):
                    last = (n == N_PER_CORE - 1 and gi == len(GROUPS) - 1
                            and half == 1)
                    do_group(n, U, t0, t1, half, split_dma=last)
            for half in range(2):
                do_direct(n, xs, half)
    nc.compile()
    return nc


# F(2,3) weight transform G (exact in bf16/fp8 for +-1 weights)
_G = np.array([[1, 0, 0], [0.5, 0.5, 0.5], [0.5, -0.5, 0.5], [0, 0, 1]],
              dtype=np.float32)


def host_weight_layout(weight: np.ndarray):
    """[256, 128, 3, 3] -> binarize; G-transform along dh for the
    Winograd rows -> [half, i, a, dw, o] fp8; plain binarized
    [half, dh, i, dw, o] fp8 for the direct rows."""
    import ml_dtypes
    wc = np.clip(weight.astype(np.float32), -1.0, 1.0)
    wbin = np.where(wc >= 0, 1.0, -1.0).astype(np.float32)
    wtr = np.einsum("ad,oidw->aoiw", _G, wbin)     # [a, o, i, dw]
    w5 = wtr.reshape(4, 2, 128, C_IN, 3)           # [a, half, oo, i, dw]
    w6 = w5.transpose(1, 3, 0, 4, 2)               # [half, i, a, dw, oo]
    wt = np.ascontiguousarray(w6).astype(ml_dtypes.float8_e4m3fn)
    wdir = wbin.reshape(2, 128, C_IN, 3, 3)        # [half, oo, i, dh, dw]
    wdir = wdir.transpose(2, 0, 3, 4, 1)           # [i, half, dh, dw, oo]
    wd = np.ascontiguousarray(wdir).astype(ml_dtypes.float8_e4m3fn)
    return wt, wd


def run(x, weight, bias, trace=False):
    """Returns (out [32,256,56,56] f32, BassKernelResults)."""
    import ml_dtypes
    nc = build_program()
    xb = np.asarray(x, dtype=np.float32).astype(ml_dtypes.bfloat16)
    wt, wd = host_weight_layout(np.asarray(weight))
    bias = np.ascontiguousarray(np.asarray(bias), dtype=np.float32)
    in_maps = [
        {
            "x": xb[i * N_PER_CORE:(i + 1) * N_PER_CORE],
            "wt": wt,
            "wd": wd,
            "b": bias,
        }
        for i in range(N_CORES)
    ]
    res = run_bass_kernel_spmd(
        nc, in_maps, core_ids=list(range(N_CORES)), trace=trace
    )
    out = np.concatenate([r["y"] for r in res.results], axis=0)
    return out.astype(np.float32), res


def kernel(x, weight, bias):
    out, _ = run(x, weight, bias)
    return out


# revision 10
# speedup vs baseline: 1.0423x; 1.0423x over previous
"""BinaryConv2d (3x3, stride 1, pad 1) on 8 TRN2 NeuronCores.

Data-parallel: batch 32 sharded 4-per-core; weight/bias replicated.

Algorithm: 1-D Winograd F(2,3) along H for rows 0..51 + direct conv for
rows 52..55. For each pair of output rows (tile row t) the conv needs 4
H-transformed input rows
  U0 = x[2t-1] - x[2t+1]   U1 = x[2t] + x[2t+1]
  U2 = x[2t+1] - x[2t]     U3 = x[2t] - x[2t+2]
and 4 transformed weight sets Wt[a][o,i,dw] = sum_dh G[a,dh] w[o,i,dh,dw]
(entries +-0.5/+-1.5/+-1, bf16-exact for binarized weights). Then
  V[a] = sum_dw Wt[a][:,:,dw] @ U[a] shifted by dw   (3 matmuls, PSUM)
  y[2t]   = V0 + V1 + V2 + bias
  y[2t+1] = V1 - V2 - V3 + bias
12 matmuls per 14 output rows per 128-channel half instead of the direct
conv's 18 -- 1.5x less PE work.

Why the direct-conv rows: measured engine occupancy shows PE/DVE/ACT all
~72-76us busy -- the kernel is 4-way balanced, not PE-bound. Winograd's
combine costs 4 DVE + 2 ACT PSUM-reading ops per group-half (one-PSUM-
operand-per-op limit) and that work CANNOT move to GpSimd (no PSUM
access). Direct-conv rows need ~zero vector work (PE accumulates all 9
taps in PSUM, one ACT bias-copy moves the tile out) at 1.5x the PE
cycles, so shifting 4 rows/image to direct trades ~5us of DVE+ACT for
~2us of PE -- converging the walls.

Width padding is avoided entirely: the dw=1 (center) tap runs first at
full width with start=True, then the dw=0/2 taps accumulate into
column-shifted PSUM windows -- the skipped edge column is exactly the
tap's zero-pad contribution.

Engine split:
 - PE: 12 MMs per (group, half) at N=392/336; 9 direct MMs per image-
   half at N=224; LDWEIGHTS hides under the MM spacing. Warmup dummy
   matmuls (gated only on a DVE memset, not GpSimd's ~6us preamble)
   bridge the fill window so the HAM clock-gate lifts early.
 - DVE: the 4 combine ops per (group, half) + image 0's U-transform
   pieces + warm_w memset.
 - GpSimd: U transforms for images 1..3 (staged two images ahead) +
   late-needed weight DMAs (half1, direct weights, bias).
 - ACT: bias folding (c0 = V0+b, c3 = b-V3, PSUM reads), direct-path
   bias copies (alternating with DVE), xs edge memsets.
 - Sync: half0 weight DMAs first (MM #1's gate, ~0.6us HWDGE issue),
   then image DMAs (contiguous 6272B/partition into the H-padded slab;
   image 0 in 4 row-chunks), all output DMAs.

Output is fp16 (halves the 12.8MB/core output DMA; host upcasts;
measured rel err 3.0e-3 vs the 2e-2 budget).
"""

import numpy as np
from contextlib import ExitStack

import concourse.bass as bass
import concourse.bacc as bacc
import concourse.mybir as mybir
import concourse.tile as tile
from concourse.bass_utils import run_bass_kernel_spmd

N_CORES = 8
N_BATCH = 32
N_PER_CORE = N_BATCH // N_CORES  # 4
C_IN = 128
C_OUT = 256
H = W = 56
HP = H + 2           # zero-padded height (in xs)
R_WINO = 52          # rows 0..51 via Winograd, 52..55 direct
T_IMG = R_WINO // 2  # 26 tile rows per image
GROUPS = ((0, 7), (7, 14), (14, 20), (20, 26))  # tile-row ranges
R_DIR = H - R_WINO   # 4 direct rows

f32 = mybir.dt.float32
bf16 = mybir.dt.bfloat16
f16 = mybir.dt.float16
fp8 = mybir.dt.float8e4
AF = mybir.ActivationFunctionType


def build_program() -> bass.Bass:
    nc = bacc.Bacc("TRN2", target_bir_lowering=False, debug=False)
    x = nc.dram_tensor("x", [N_PER_CORE, C_IN, H, W], bf16, kind="ExternalInput")
    # wt[half, i, a, dw, o]: host-transformed Winograd weights, fp8-exact.
    wt = nc.dram_tensor("wt", [2, C_IN, 4, 3, 128], fp8, kind="ExternalInput")
    # wd[i, half, dh, dw, o]: plain binarized weights for the direct rows.
    wd = nc.dram_tensor("wd", [C_IN, 2, 3, 3, 128], fp8, kind="ExternalInput")
    b = nc.dram_tensor("b", [C_OUT], f32, kind="ExternalInput")
    y = nc.dram_tensor("y", [N_PER_CORE, C_OUT, H, W], f16, kind="ExternalOutput")

    with tile.TileContext(nc) as tc, ExitStack() as ctx:
        singles = ctx.enter_context(tc.tile_pool(name="singles", bufs=1))
        xsp = ctx.enter_context(tc.tile_pool(name="xsp", bufs=4))
        up = ctx.enter_context(tc.tile_pool(name="up", bufs=3))
        psum_mm = ctx.enter_context(
            tc.tile_pool(name="psum_mm", bufs=8, space="PSUM")
        )
        tdp = ctx.enter_context(tc.tile_pool(name="tdp", bufs=8))
        obp = ctx.enter_context(tc.tile_pool(name="obp", bufs=4))

        # warmup weights come from the DVE (fast queue start); warmup MMs
        # then only gate on DVE, not GpSimd's ~6us preamble
        warm_w = singles.tile([128, 128], bf16)
        nc.vector.memset(warm_w, 0.0)

        wtile = singles.tile([128, 2, 4, 3, 128], fp8, name="wt")
        wdt = singles.tile([128, 2, 3, 3, 128], fp8, name="wd")
        bsb = singles.tile([128, 2], f32)

        def stage_image(n):
            """One contiguous DMA into the H-padded slab + edge memsets +
            the four H-transform ops (GpSimd; staged 2 images ahead)."""
            xs = xsp.tile([128, HP, W], bf16, name="xs")
            nc.sync.dma_start(out=xs[:, 1:1 + H, :], in_=x.ap()[n])
            nc.scalar.memzero(xs[:, 0, :])
            nc.scalar.memzero(xs[:, HP - 1, :])

            def e(k):  # rows k, k+2, ..., k+50 of the padded slab
                return xs[:, k:k + 2 * (T_IMG - 1) + 1:2, :]

            U = [up.tile([128, T_IMG, W], bf16, name=f"u{a}", tag=f"u{a}")
                 for a in range(4)]
            nc.gpsimd.tensor_sub(U[0], e(0), e(2))
            nc.gpsimd.tensor_add(U[1], e(1), e(2))
            nc.gpsimd.tensor_sub(U[2], e(2), e(1))
            nc.gpsimd.tensor_sub(U[3], e(1), e(3))
            return xs, U

        def stage_image0():
            """Image 0 rides the critical path: half0 weights issue from
            Sync FIRST (HWDGE ~0.6us issue; MM #1 gates on 196KB), then
            4 row-chunk image DMAs; late-needed weights (half1, direct,
            bias) issue from the GpSimd queue once its preamble ends."""
            xs = xsp.tile([128, HP, W], bf16, name="xs")
            nc.gpsimd.dma_start(out=wtile[:, 0, 0], in_=wt.ap()[0][:, 0])
            nc.gpsimd.dma_start(out=wtile[:, 0, 1:4], in_=wt.ap()[0][:, 1:4])
            nc.gpsimd.dma_start(out=wtile[:, 1], in_=wt.ap()[1])
            nc.gpsimd.dma_start(
                out=bsb, in_=b.ap().rearrange("(h o) -> o h", h=2))
            nc.gpsimd.dma_start(out=wdt, in_=wd.ap())
            chunks = [(0, 16), (16, 30), (30, 42), (42, 56)]
            for lo, hi in chunks:
                nc.sync.dma_start(out=xs[:, 1 + lo:1 + hi, :],
                                  in_=x.ap()[0, :, lo:hi, :])
            nc.scalar.memzero(xs[:, 0, :])
            nc.scalar.memzero(xs[:, HP - 1, :])

            U = [up.tile([128, T_IMG, W], bf16, name=f"u{a}", tag=f"u{a}")
                 for a in range(4)]

            def piece(t0, t1, eng):
                r = slice(t0, t1)

                def e(k):
                    lo = 2 * t0 + k
                    return xs[:, lo:lo + 2 * (t1 - t0 - 1) + 1:2, :]

                eng.tensor_sub(U[0][:, r, :], e(0), e(2))
                eng.tensor_add(U[1][:, r, :], e(1), e(2))
                eng.tensor_sub(U[2][:, r, :], e(2), e(1))
                eng.tensor_sub(U[3][:, r, :], e(1), e(3))

            # pieces 0-2 on DVE (fast, ahead of the combine backlog),
            # the last on GpSimd
            for (t0, t1) in GROUPS[:-1]:
                piece(t0, t1, nc.vector)
            piece(*GROUPS[-1], nc.gpsimd)
            return xs, U

        # ---- PE warmup: bridge the pipeline-fill window with dummy
        # matmuls so the HAM clock-gate (K=4/8 cold throttle) lifts
        # before the real stream starts.
        wp = psum_mm.tile([128, 128], f32, tag="ps")
        NWARM = 36
        for k in range(NWARM):
            nc.tensor.matmul(wp, lhsT=warm_w, rhs=warm_w,
                             start=(k == 0), stop=(k == NWARM - 1))

        def do_group(n, U, t0, t1, half, split_dma=False):
            """Output rows 2*t0..2*t1 of image n, one half."""
            tg = t1 - t0
            h0 = 2 * t0
            r = slice(t0, t1)
            nw = tg * W
            V = [None] * 4
            # last group only: a=3 first, so its bias-fold (c3) runs during
            # the remaining matmuls and only y0/y1 trail the last one
            for a in ((3, 0, 1, 2) if split_dma else (0, 1, 2, 3)):
                ps = psum_mm.tile([128, tg, W], f32, name=f"v{a}",
                                  tag="ps")
                lt = wtile[:, half, a]
                # center tap first at full width (sets has_written), then
                # the shifted taps accumulate into column-shifted windows
                nc.tensor.matmul(ps, lhsT=lt[:, 1], rhs=U[a][:, r, :],
                                 start=True, stop=False)
                nc.tensor.matmul(ps[:, :, 1:W], lhsT=lt[:, 0],
                                 rhs=U[a][:, r, 0:W - 1],
                                 start=False, stop=False)
                nc.tensor.matmul(ps[:, :, 0:W - 1], lhsT=lt[:, 2],
                                 rhs=U[a][:, r, 1:W],
                                 start=False, stop=True)
                V[a] = ps
            # y0 = V0+V1+V2+b, y1 = V1-V2-V3+b; ACT folds the bias into the
            # single-use terms, DVE does the two-tensor combines (max one
            # PSUM operand each).
            ob = obp.tile([128, tg, 2, W], f16, name="ob", tag="ob")
            c0 = tdp.tile([128, tg, W], f32, name="c0", tag="td")
            c3 = tdp.tile([128, tg, W], f32, name="c3", tag="td")
            t = tdp.tile([128, tg, W], f32, name="t", tag="td")
            e = tdp.tile([128, tg, W], f32, name="e", tag="td")

            def act_c0():
                nc.scalar.activation(c0, V[0], AF.Identity,
                                     bias=bsb[:, half:half + 1])

            def act_c3():
                nc.scalar.activation(c3, V[3], AF.Identity,
                                     bias=bsb[:, half:half + 1], scale=-1.0)

            ych = y.ap()[n, half * 128:(half + 1) * 128]
            if split_dma:
                # tail: c3/c0/t/e all complete during the a=0..2 matmuls
                # (a=3 ran first), so only y0/y1 trail the last matmul --
                # and they go in two row-blocks so the first block's DMA
                # overlaps the second block's compute.
                act_c3(), act_c0()
                nc.vector.tensor_add(t, c0, V[1])
                nc.vector.tensor_add(e, c3, V[1])
                for rlo, rhi in ((0, 3), (3, tg)):
                    nc.vector.tensor_add(ob[:, rlo:rhi, 0, :],
                                         t[:, rlo:rhi], V[2][:, rlo:rhi])
                    nc.vector.tensor_sub(ob[:, rlo:rhi, 1, :],
                                         e[:, rlo:rhi], V[2][:, rlo:rhi])
                    nc.sync.dma_start(
                        out=ych[:, h0 + 2 * rlo:h0 + 2 * rhi, :],
                        in_=ob[:, rlo:rhi],
                    )
            else:
                act_c0()
                nc.vector.tensor_add(t, c0, V[1])
                nc.vector.tensor_add(ob[:, :, 0, :], t, V[2])
                act_c3()
                nc.vector.tensor_add(e, c3, V[1])
                nc.vector.tensor_sub(ob[:, :, 1, :], e, V[2])
                nc.sync.dma_start(out=ych[:, h0:h0 + 2 * tg, :], in_=ob)

        def do_direct(n, xs, half):
            """Rows 52..55 of image n, one half: direct conv, all 9 taps
            accumulated in PSUM; one bias-copy (alternating ACT/DVE to
            balance load) moves the tile out."""
            ps = psum_mm.tile([128, R_DIR, W], f32, name="vd",
                              tag="ps")
            first = True
            for dh in range(3):
                # output rows 52..55 <- x rows 51+dh..54+dh = xs rows 52+dh..
                rx = xs[:, R_WINO + dh:R_WINO + dh + R_DIR, :]
                lt = wdt[:, half, dh]
                nc.tensor.matmul(ps, lhsT=lt[:, 1], rhs=rx,
                                 start=first, stop=False)
                first = False
                nc.tensor.matmul(ps[:, :, 1:W], lhsT=lt[:, 0],
                                 rhs=rx[:, :, 0:W - 1],
                                 start=False, stop=False)
                nc.tensor.matmul(ps[:, :, 0:W - 1], lhsT=lt[:, 2],
                                 rhs=rx[:, :, 1:W],
                                 start=False, stop=(dh == 2))
            ob = obp.tile([128, R_DIR, W], f16, name="obd", tag="obd")
            bias = bsb[:, half:half + 1]
            if (n + half) % 2 == 0:
                nc.scalar.activation(ob, ps, AF.Identity, bias=bias)
            else:
                nc.vector.tensor_scalar(out=ob, in0=ps, scalar1=1.0,
                                        scalar2=bias,
                                        op0=mybir.AluOpType.mult,
                                        op1=mybir.AluOpType.add)
            ych = y.ap()[n, half * 128:(half + 1) * 128]
            nc.sync.dma_start(out=ych[:, R_WINO:H, :], in_=ob)

        # software pipeline: staged two images ahead so GpSimd's slower
        # transform rate never gates the PE
        imgs = [stage_image0(), stage_image(1), stage_image(2)]
        for n in range(N_PER_CORE):
            if n + 3 < N_PER_CORE:
                imgs.append(stage_image(n + 3))
            xs, U = imgs[n]
            for gi, (t0, t1) in enumerate(GROUPS):
                for half in range(2):
                    last = (n == N_PER_CORE - 1 and gi == len(GROUPS) - 1
                            and half == 1)
                    do_group(n, U, t0, t1, half, split_dma=last)
            for half in range(2):
                do_direct(n, xs, half)
    nc.compile()
    return nc


# F(2,3) weight transform G (exact in bf16/fp8 for +-1 weights)
_G = np.array([[1, 0, 0], [0.5, 0.5, 0.5], [0.5, -0.5, 0.5], [0, 0, 1]],
              dtype=np.float32)


def host_weight_layout(weight: np.ndarray):
    """[256, 128, 3, 3] -> binarize; G-transform along dh for the
    Winograd rows -> [half, i, a, dw, o] fp8; plain binarized
    [half, dh, i, dw, o] fp8 for the direct rows."""
    import ml_dtypes
    wc = np.clip(weight.astype(np.float32), -1.0, 1.0)
    wbin = np.where(wc >= 0, 1.0, -1.0).astype(np.float32)
    wtr = np.einsum("ad,oidw->aoiw", _G, wbin)     # [a, o, i, dw]
    w5 = wtr.reshape(4, 2, 128, C_IN, 3)           # [a, half, oo, i, dw]
    w6 = w5.transpose(1, 3, 0, 4, 2)               # [half, i, a, dw, oo]
    wt = np.ascontiguousarray(w6).astype(ml_dtypes.float8_e4m3fn)
    wdir = wbin.reshape(2, 128, C_IN, 3, 3)        # [half, oo, i, dh, dw]
    wdir = wdir.transpose(2, 0, 3, 4, 1)           # [i, half, dh, dw, oo]
    wd = np.ascontiguousarray(wdir).astype(ml_dtypes.float8_e4m3fn)
    return wt, wd


def run(x, weight, bias, trace=False):
    """Returns (out [32,256,56,56] f32, BassKernelResults)."""
    import ml_dtypes
    nc = build_program()
    xb = np.asarray(x, dtype=np.float32).astype(ml_dtypes.bfloat16)
    wt, wd = host_weight_layout(np.asarray(weight))
    bias = np.ascontiguousarray(np.asarray(bias), dtype=np.float32)
    in_maps = [
        {
            "x": xb[i * N_PER_CORE:(i + 1) * N_PER_CORE],
            "wt": wt,
            "wd": wd,
            "b": bias,
        }
        for i in range(N_CORES)
    ]
    res = run_bass_kernel_spmd(
        nc, in_maps, core_ids=list(range(N_CORES)), trace=trace
    )
    out = np.concatenate([r["y"] for r in res.results], axis=0)
    return out.astype(np.float32), res


def kernel(x, weight, bias):
    out, _ = run(x, weight, bias)
    return out


# revision 11
# speedup vs baseline: 1.0640x; 1.0208x over previous
"""BinaryConv2d (3x3, stride 1, pad 1) on 8 TRN2 NeuronCores.

Data-parallel: batch 32 sharded 4-per-core; weight/bias replicated.

Algorithm: 1-D Winograd F(2,3) along H. For each pair of output rows
(tile row t) the conv needs 4 H-transformed input rows
  U0 = x[2t-1] - x[2t+1]   U1 = x[2t] + x[2t+1]
  U2 = x[2t+1] - x[2t]     U3 = x[2t] - x[2t+2]
and 4 transformed weight sets Wt[a][o,i,dw] = sum_dh G[a,dh] w[o,i,dh,dw]
(entries +-0.5/+-1.5/+-1, bf16-exact for binarized weights). Then
  V[a] = sum_dw Wt[a][:,:,dw] @ U[a] shifted by dw   (3 matmuls, PSUM)
  y[2t]   = V0 + V1 + V2 + bias
  y[2t+1] = V1 - V2 - V3 + bias
12 matmuls per 14 output rows per 128-channel half instead of the direct
conv's 18 -- 1.5x less PE work, and PE is the bottleneck.

Width padding is avoided entirely: the dw=1 (center) tap runs first at
full width N=392 with start=True, then the dw=0/2 taps accumulate into
column-shifted PSUM windows at N=385 -- the skipped edge column is
exactly the tap's zero-pad contribution.

Engine split (arrived at over ~10 traced iterations):
 - PE: 12 MMs per (group, half) at N=392; weights resident in SBUF, the
   LDWEIGHTS for each hides under the 166ns MM spacing (measured: warm
   steady-state spacing == the 392-cycle streaming floor, zero gaps).
 - DVE: the 4 unavoidable two-tensor output combines per (group, half)
   (max one PSUM operand each -- hw limit) + image 0's input transform
   (computed per 7-tile-row piece so MM #1 can start ~10.5us in).
 - GpSimd: input transform for images 1..3 (staged two images ahead so
   its ~3.8us/op rate stays off the critical path) + the weight/bias DMA
   issues (its queue is idle early; DMA issue ops cost ~650ns each and
   would serialize behind the image chunks on Sync).
 - ACT: bias folding (c0 = V0+b, c3 = b-V3, reading PSUM) + xs edge
   memsets.
 - Sync: image DMAs -- fully-contiguous 6272B/partition transfers into
   an H-padded-only slab (a W-padded slab forces 112B-chunk DMA at 1/4
   rate); image 0 is split into 4 row-chunks so the first transform
   piece gates on 229KB only; all output DMAs.
 - PE warmup: 36 dummy matmuls bridge the preamble+fill window so the
   HAM clock-gate (K=4/8 cold throttle) lifts before the real stream.

Output is written fp16 (host upcasts to f32): halves the 12.8MB/core
output DMA -- with all 8 cores streaming results simultaneously this
cuts chip-wide HBM write pressure ~in half, and the fp16 rounding of
|y|<~190 adds only ~3e-4 rel (measured total 3.04e-3 vs budget 2e-2).

Measured variants that LOST and why (for future reference): hybrid
direct-conv rows (PE stream is the wall's backbone; +9 direct MMs/image
lengthened it more than the DVE relief shortened the tail); F(4,3)
Winograd (PE drops to 47us but the 18-op inverse transform exceeds
DVE+ACT+GpSimd capacity -- GpSimd cannot read PSUM and runs ~2.4ns/elem);
fp8 DoubleRow (activation quantization alone costs 3.7e-2 rel, over the
2e-2 gate); finer tail splits (each extra output DMA costs ~590ns of
issue time on the Sync queue); smaller first group (mixed PSUM tile
sizes / extra small ops cost more than the earlier stream start saved).
"""

import numpy as np
from contextlib import ExitStack

import concourse.bass as bass
import concourse.bacc as bacc
import concourse.mybir as mybir
import concourse.tile as tile
from concourse.bass_utils import run_bass_kernel_spmd

N_CORES = 8
N_BATCH = 32
N_PER_CORE = N_BATCH // N_CORES  # 4
C_IN = 128
C_OUT = 256
H = W = 56
HP = H + 2           # zero-padded height (in xs)
T_IMG = H // 2       # 28 tile rows per image
T_GRP = 7            # tile rows per matmul group
NGRP = T_IMG // T_GRP  # 4 groups -> 14 output rows each

f32 = mybir.dt.float32
bf16 = mybir.dt.bfloat16
f16 = mybir.dt.float16
AF = mybir.ActivationFunctionType


def build_program() -> bass.Bass:
    nc = bacc.Bacc("TRN2", target_bir_lowering=False, debug=False)
    x = nc.dram_tensor("x", [N_PER_CORE, C_IN, H, W], bf16, kind="ExternalInput")
    # wt[half, i, a, dw, o]: host-transformed Winograd weights. The values
    # (0, +-0.5, +-1, +-1.5) are exact in fp8-e4m3, which halves the
    # weight-stream bytes (the early-fill gate) at zero accuracy cost;
    # a single-fp8 lhsT against a bf16 rhs runs at bf16 speed.
    fp8 = mybir.dt.float8e4
    wt = nc.dram_tensor("wt", [2, C_IN, 4, 3, 128], fp8, kind="ExternalInput")
    b = nc.dram_tensor("b", [C_OUT], f32, kind="ExternalInput")
    y = nc.dram_tensor("y", [N_PER_CORE, C_OUT, H, W], f16, kind="ExternalOutput")

    with tile.TileContext(nc) as tc, ExitStack() as ctx:
        singles = ctx.enter_context(tc.tile_pool(name="singles", bufs=1))
        xsp = ctx.enter_context(tc.tile_pool(name="xsp", bufs=4))
        up = ctx.enter_context(tc.tile_pool(name="up", bufs=3))
        psum_mm = ctx.enter_context(
            tc.tile_pool(name="psum_mm", bufs=8, space="PSUM")
        )
        tdp = ctx.enter_context(tc.tile_pool(name="tdp", bufs=8))
        obp = ctx.enter_context(tc.tile_pool(name="obp", bufs=4))

        # warmup weight tile first: its GpSimd memset is the earliest
        # producer any PE work can gate on
        warm_w = singles.tile([128, 128], bf16)
        nc.gpsimd.memset(warm_w, 0.0)

        wtile = singles.tile([128, 2, 4, 3, 128], mybir.dt.float8e4,
                             name="wt")
        bsb = singles.tile([128, 2], f32)

        def stage_image(n):
            """One contiguous DMA into the H-padded slab + edge memsets +
            the four H-transform ops (GpSimd; staged 2 images ahead)."""
            xs = xsp.tile([128, HP, W], bf16, name="xs")
            nc.sync.dma_start(out=xs[:, 1:1 + H, :], in_=x.ap()[n])
            nc.scalar.memzero(xs[:, 0, :])
            nc.scalar.memzero(xs[:, HP - 1, :])

            def e(k):  # rows k, k+2, ..., k+54 of the padded slab
                return xs[:, k:k + 2 * (T_IMG - 1) + 1:2, :]

            U = [up.tile([128, T_IMG, W], bf16, name=f"u{a}", tag=f"u{a}")
                 for a in range(4)]
            nc.gpsimd.tensor_sub(U[0], e(0), e(2))
            nc.gpsimd.tensor_add(U[1], e(1), e(2))
            nc.gpsimd.tensor_sub(U[2], e(2), e(1))
            nc.gpsimd.tensor_sub(U[3], e(1), e(3))
            return U

        def stage_image0():
            """Image 0 rides the critical path: 4 row-chunk DMAs (first
            matmul gates on a 229KB transfer, not 784KB) + per-group
            U-transform pieces on the DVE; weights/bias issue from the
            GpSimd queue in parallel, (half0, a0) first."""
            xs = xsp.tile([128, HP, W], bf16, name="xs")
            # weights + bias issue from the (idle) GpSimd queue so their
            # ~650ns issue ops don't serialize behind the image chunks
            # on the Sync queue; (half0, a0) goes alone so MM #1's gate
            # is a 196KB transfer
            nc.gpsimd.dma_start(out=wtile[:, 0, 0], in_=wt.ap()[0][:, 0])
            nc.gpsimd.dma_start(out=wtile[:, 0, 1:4], in_=wt.ap()[0][:, 1:4])
            nc.gpsimd.dma_start(out=wtile[:, 1], in_=wt.ap()[1])
            nc.gpsimd.dma_start(
                out=bsb, in_=b.ap().rearrange("(h o) -> o h", h=2))
            chunks = [(0, 16), (16, 30), (30, 44), (44, 56)]
            for lo, hi in chunks:
                nc.sync.dma_start(out=xs[:, 1 + lo:1 + hi, :],
                                  in_=x.ap()[0, :, lo:hi, :])
            nc.scalar.memzero(xs[:, 0, :])
            nc.scalar.memzero(xs[:, HP - 1, :])

            U = [up.tile([128, T_IMG, W], bf16, name=f"u{a}", tag=f"u{a}")
                 for a in range(4)]

            def piece(g, eng):
                r = slice(T_GRP * g, T_GRP * (g + 1))

                def e(k):  # rows 14g+k, +2, ..., +12 of the padded slab
                    return xs[:, 14 * g + k:14 * g + k + 13:2, :]

                eng.tensor_sub(U[0][:, r, :], e(0), e(2))
                eng.tensor_add(U[1][:, r, :], e(1), e(2))
                eng.tensor_sub(U[2][:, r, :], e(2), e(1))
                eng.tensor_sub(U[3][:, r, :], e(1), e(3))

            # pieces 0-2 on DVE (fast, ahead of the output-op backlog),
            # the last on GpSimd (idle until image 1's transform; its
            # ~1us/op rate still beats group 3's ~21us deadline)
            for g in range(NGRP - 1):
                piece(g, nc.vector)
            piece(NGRP - 1, nc.gpsimd)
            return U

        # ---- PE warmup: bridge the pipeline-fill window (preamble + first
        # image DMA + first U transform) with dummy matmuls so the HAM
        # clock-gate lifts before the real stream starts.
        wp = psum_mm.tile([128, 128], f32, tag="ps")
        NWARM = 36
        for k in range(NWARM):
            nc.tensor.matmul(wp, lhsT=warm_w, rhs=warm_w,
                             start=(k == 0), stop=(k == NWARM - 1))

        def do_group(n, U, g, half, split_dma=False):
            """14 output rows (tile rows 7g..7g+6) of image n, one half."""
            h0 = 2 * T_GRP * g
            r = slice(T_GRP * g, T_GRP * (g + 1))
            V = [None] * 4
            # last group only: a=3 first, so its bias-fold (c3) runs during
            # the remaining matmuls and only y0/y1 trail the last one
            for a in ((3, 0, 1, 2) if split_dma else (0, 1, 2, 3)):
                ps = psum_mm.tile([128, T_GRP, W], f32, name=f"v{a}",
                                  tag="ps")
                lt = wtile[:, half, a]
                # center tap first at full width (sets has_written), then
                # the shifted taps accumulate into partial column windows
                nc.tensor.matmul(ps, lhsT=lt[:, 1], rhs=U[a][:, r, :],
                                 start=True, stop=False)
                nc.tensor.matmul(ps[:, :, 1:W], lhsT=lt[:, 0],
                                 rhs=U[a][:, r, 0:W - 1],
                                 start=False, stop=False)
                nc.tensor.matmul(ps[:, :, 0:W - 1], lhsT=lt[:, 2],
                                 rhs=U[a][:, r, 1:W],
                                 start=False, stop=True)
                V[a] = ps
            # y0 = V0+V1+V2+b, y1 = V1-V2-V3+b; ACT folds the bias into the
            # single-use terms, DVE does the two-tensor combines (max one
            # PSUM operand each).
            ob = obp.tile([128, T_GRP, 2, W], f16, name="ob", tag="ob")
            c0 = tdp.tile([128, T_GRP, W], f32, name="c0", tag="td")
            c3 = tdp.tile([128, T_GRP, W], f32, name="c3", tag="td")
            t = tdp.tile([128, T_GRP, W], f32, name="t", tag="td")
            e = tdp.tile([128, T_GRP, W], f32, name="e", tag="td")

            def act_c0():
                nc.scalar.activation(c0, V[0], AF.Identity,
                                     bias=bsb[:, half:half + 1])

            def act_c3():
                nc.scalar.activation(c3, V[3], AF.Identity,
                                     bias=bsb[:, half:half + 1], scale=-1.0)

            ych = y.ap()[n, half * 128:(half + 1) * 128]
            if split_dma:
                # tail: c3/c0/t/e all complete during the a=0..2 matmuls
                # (a=3 ran first), so only y0/y1 trail the last matmul --
                # and they go in two row-blocks so the first block's DMA
                # overlaps the second block's compute. All transfers stay
                # row-contiguous.
                act_c3(), act_c0()
                nc.vector.tensor_add(t, c0, V[1])
                nc.vector.tensor_add(e, c3, V[1])
                for rlo, rhi in ((0, 4), (4, T_GRP)):
                    nc.vector.tensor_add(ob[:, rlo:rhi, 0, :],
                                         t[:, rlo:rhi], V[2][:, rlo:rhi])
                    nc.vector.tensor_sub(ob[:, rlo:rhi, 1, :],
                                         e[:, rlo:rhi], V[2][:, rlo:rhi])
                    nc.sync.dma_start(
                        out=ych[:, h0 + 2 * rlo:h0 + 2 * rhi, :],
                        in_=ob[:, rlo:rhi],
                    )
            else:
                act_c0()
                nc.vector.tensor_add(t, c0, V[1])
                nc.vector.tensor_add(ob[:, :, 0, :], t, V[2])
                act_c3()
                nc.vector.tensor_add(e, c3, V[1])
                nc.vector.tensor_sub(ob[:, :, 1, :], e, V[2])
                nc.sync.dma_start(out=ych[:, h0:h0 + 2 * T_GRP, :], in_=ob)

        # software pipeline: staged two images ahead so GpSimd's slower
        # transform rate never gates the PE
        Us = [stage_image0(), stage_image(1), stage_image(2)]
        for n in range(N_PER_CORE):
            if n + 3 < N_PER_CORE:
                Us.append(stage_image(n + 3))
            for g in range(NGRP):
                for half in range(2):
                    last = (n == N_PER_CORE - 1 and g == NGRP - 1
                            and half == 1)
                    do_group(n, Us[n], g, half, split_dma=last)
    nc.compile()
    return nc


# F(2,3) weight transform G (exact in bf16 for +-1 weights)
_G = np.array([[1, 0, 0], [0.5, 0.5, 0.5], [0.5, -0.5, 0.5], [0, 0, 1]],
              dtype=np.float32)


def host_weight_layout(weight: np.ndarray) -> np.ndarray:
    """[256, 128, 3, 3] -> binarize, G-transform along dh,
    layout [half, i, a, dw, o] = [2, 128, 4, 3, 128] fp8-e4m3
    (values 0/+-0.5/+-1/+-1.5 are e4m3-exact)."""
    import ml_dtypes
    wc = np.clip(weight.astype(np.float32), -1.0, 1.0)
    wbin = np.where(wc >= 0, 1.0, -1.0).astype(np.float32)
    wtr = np.einsum("ad,oidw->aoiw", _G, wbin)     # [a, o, i, dw]
    w5 = wtr.reshape(4, 2, 128, C_IN, 3)           # [a, half, oo, i, dw]
    w6 = w5.transpose(1, 3, 0, 4, 2)               # [half, i, a, dw, oo]
    return np.ascontiguousarray(w6).astype(ml_dtypes.float8_e4m3fn)


def run(x, weight, bias, trace=False):
    """Returns (out [32,256,56,56] f32, BassKernelResults)."""
    import ml_dtypes
    nc = build_program()
    xb = np.asarray(x, dtype=np.float32).astype(ml_dtypes.bfloat16)
    wtr = host_weight_layout(np.asarray(weight))
    bias = np.ascontiguousarray(np.asarray(bias), dtype=np.float32)
    in_maps = [
        {
            "x": xb[i * N_PER_CORE:(i + 1) * N_PER_CORE],
            "wt": wtr,
            "b": bias,
        }
        for i in range(N_CORES)
    ]
    res = run_bass_kernel_spmd(
        nc, in_maps, core_ids=list(range(N_CORES)), trace=trace
    )
    out = np.concatenate([r["y"] for r in res.results], axis=0)
    return out.astype(np.float32), res


def kernel(x, weight, bias):
    out, _ = run(x, weight, bias)
    return out



# revision 12
# speedup vs baseline: 1.0744x; 1.0098x over previous
"""BinaryConv2d (3x3, stride 1, pad 1) on 8 TRN2 NeuronCores.

Data-parallel: batch 32 sharded 4-per-core; weight/bias replicated.

Algorithm: 1-D Winograd F(2,3) along H. For each pair of output rows
(tile row t) the conv needs 4 H-transformed input rows
  U0 = x[2t-1] - x[2t+1]   U1 = x[2t] + x[2t+1]
  U2 = x[2t+1] - x[2t]     U3 = x[2t] - x[2t+2]
and 4 transformed weight sets Wt[a][o,i,dw] = sum_dh G[a,dh] w[o,i,dh,dw]
(entries +-0.5/+-1.5/+-1, bf16-exact for binarized weights). Then
  V[a] = sum_dw Wt[a][:,:,dw] @ U[a] shifted by dw   (3 matmuls, PSUM)
  y[2t]   = V0 + V1 + V2 + bias
  y[2t+1] = V1 - V2 - V3 + bias
12 matmuls per 14 output rows per 128-channel half instead of the direct
conv's 18 -- 1.5x less PE work, and PE is the bottleneck.

Width padding is avoided entirely: the dw=1 (center) tap runs first at
full width N=392 with start=True, then the dw=0/2 taps accumulate into
column-shifted PSUM windows at N=385 -- the skipped edge column is
exactly the tap's zero-pad contribution.

Engine split (arrived at over ~10 traced iterations):
 - PE: 12 MMs per (group, half) at N=392; weights resident in SBUF, the
   LDWEIGHTS for each hides under the 166ns MM spacing (measured: warm
   steady-state spacing == the 392-cycle streaming floor, zero gaps).
 - DVE: the 4 unavoidable two-tensor output combines per (group, half)
   (max one PSUM operand each -- hw limit) + ALL of image 0's input
   transform pieces (per 7-tile-row piece so MM #1 can start ~10.5us
   in; keeping piece 3 off GpSimd starts image 1's transform ~8us
   earlier, protecting the image-boundary margin when the chip runs in
   its slow/throttled state).
 - GpSimd: input transform for images 1..3 (staged two images ahead so
   its ~3.8us/op rate stays off the critical path) + the weight/bias DMA
   issues (its queue is idle early; DMA issue ops cost ~650ns each and
   would serialize behind the image chunks on Sync).
 - ACT: bias folding (c0 = V0+b, c3 = b-V3, reading PSUM) + xs edge
   memsets.
 - Sync: image DMAs -- fully-contiguous 6272B/partition transfers into
   an H-padded-only slab (a W-padded slab forces 112B-chunk DMA at 1/4
   rate); image 0 is split into 4 row-chunks so the first transform
   piece gates on 229KB only; all output DMAs.
 - PE warmup: 36 dummy matmuls bridge the preamble+fill window so the
   HAM clock-gate (K=4/8 cold throttle) lifts before the real stream.
"""

import numpy as np
from contextlib import ExitStack

import concourse.bass as bass
import concourse.bacc as bacc
import concourse.mybir as mybir
import concourse.tile as tile
from concourse.bass_utils import run_bass_kernel_spmd

N_CORES = 8
N_BATCH = 32
N_PER_CORE = N_BATCH // N_CORES  # 4
C_IN = 128
C_OUT = 256
H = W = 56
HP = H + 2           # zero-padded height (in xs)
T_IMG = H // 2       # 28 tile rows per image
T_GRP = 7            # tile rows per matmul group
NGRP = T_IMG // T_GRP  # 4 groups -> 14 output rows each

f32 = mybir.dt.float32
bf16 = mybir.dt.bfloat16
f16 = mybir.dt.float16
AF = mybir.ActivationFunctionType


def build_program() -> bass.Bass:
    nc = bacc.Bacc("TRN2", target_bir_lowering=False, debug=False)
    x = nc.dram_tensor("x", [N_PER_CORE, C_IN, H, W], bf16, kind="ExternalInput")
    # wt[half, i, a, dw, o]: host-transformed Winograd weights. The values
    # (0, +-0.5, +-1, +-1.5) are exact in fp8-e4m3, which halves the
    # weight-stream bytes (the early-fill gate) at zero accuracy cost;
    # a single-fp8 lhsT against a bf16 rhs runs at bf16 speed.
    fp8 = mybir.dt.float8e4
    wt = nc.dram_tensor("wt", [2, C_IN, 4, 3, 128], fp8, kind="ExternalInput")
    b = nc.dram_tensor("b", [C_OUT], f32, kind="ExternalInput")
    y = nc.dram_tensor("y", [N_PER_CORE, C_OUT, H, W], f16, kind="ExternalOutput")

    with tile.TileContext(nc) as tc, ExitStack() as ctx:
        singles = ctx.enter_context(tc.tile_pool(name="singles", bufs=1))
        xsp = ctx.enter_context(tc.tile_pool(name="xsp", bufs=4))
        up = ctx.enter_context(tc.tile_pool(name="up", bufs=3))
        psum_mm = ctx.enter_context(
            tc.tile_pool(name="psum_mm", bufs=8, space="PSUM")
        )
        tdp = ctx.enter_context(tc.tile_pool(name="tdp", bufs=8))
        obp = ctx.enter_context(tc.tile_pool(name="obp", bufs=4))

        # warmup weight tile first: its GpSimd memset is the earliest
        # producer any PE work can gate on
        warm_w = singles.tile([128, 128], bf16)
        nc.gpsimd.memset(warm_w, 0.0)

        wtile = singles.tile([128, 2, 4, 3, 128], mybir.dt.float8e4,
                             name="wt")
        bsb = singles.tile([128, 2], f32)

        def stage_image(n):
            """One contiguous DMA into the H-padded slab + edge memsets +
            the four H-transform ops (GpSimd; staged 2 images ahead)."""
            xs = xsp.tile([128, HP, W], bf16, name="xs")
            nc.sync.dma_start(out=xs[:, 1:1 + H, :], in_=x.ap()[n])
            nc.scalar.memzero(xs[:, 0, :])
            nc.scalar.memzero(xs[:, HP - 1, :])

            def e(k):  # rows k, k+2, ..., k+54 of the padded slab
                return xs[:, k:k + 2 * (T_IMG - 1) + 1:2, :]

            U = [up.tile([128, T_IMG, W], bf16, name=f"u{a}", tag=f"u{a}")
                 for a in range(4)]
            nc.gpsimd.tensor_sub(U[0], e(0), e(2))
            nc.gpsimd.tensor_add(U[1], e(1), e(2))
            nc.gpsimd.tensor_sub(U[2], e(2), e(1))
            nc.gpsimd.tensor_sub(U[3], e(1), e(3))
            return U

        def stage_image0():
            """Image 0 rides the critical path: 4 row-chunk DMAs (first
            matmul gates on a 229KB transfer, not 784KB) + per-group
            U-transform pieces on the DVE; weights/bias issue from the
            GpSimd queue in parallel, (half0, a0) first."""
            xs = xsp.tile([128, HP, W], bf16, name="xs")
            # weights + bias issue from the (idle) GpSimd queue so their
            # ~650ns issue ops don't serialize behind the image chunks
            # on the Sync queue; (half0, a0) goes alone so MM #1's gate
            # is a 196KB transfer
            nc.gpsimd.dma_start(out=wtile[:, 0, 0], in_=wt.ap()[0][:, 0])
            nc.gpsimd.dma_start(out=wtile[:, 0, 1:4], in_=wt.ap()[0][:, 1:4])
            nc.gpsimd.dma_start(out=wtile[:, 1], in_=wt.ap()[1])
            nc.gpsimd.dma_start(
                out=bsb, in_=b.ap().rearrange("(h o) -> o h", h=2))
            chunks = [(0, 16), (16, 30), (30, 44), (44, 56)]
            for lo, hi in chunks:
                nc.sync.dma_start(out=xs[:, 1 + lo:1 + hi, :],
                                  in_=x.ap()[0, :, lo:hi, :])
            nc.scalar.memzero(xs[:, 0, :])
            nc.scalar.memzero(xs[:, HP - 1, :])

            U = [up.tile([128, T_IMG, W], bf16, name=f"u{a}", tag=f"u{a}")
                 for a in range(4)]

            def piece(g, eng):
                r = slice(T_GRP * g, T_GRP * (g + 1))

                def e(k):  # rows 14g+k, +2, ..., +12 of the padded slab
                    return xs[:, 14 * g + k:14 * g + k + 13:2, :]

                eng.tensor_sub(U[0][:, r, :], e(0), e(2))
                eng.tensor_add(U[1][:, r, :], e(1), e(2))
                eng.tensor_sub(U[2][:, r, :], e(2), e(1))
                eng.tensor_sub(U[3][:, r, :], e(1), e(3))

            # pieces 0-2 on DVE (fast, ahead of the output-op backlog),
            # the last on GpSimd (idle until image 1's transform; its
            # ~1us/op rate still beats group 3's ~21us deadline)
            for g in range(NGRP):
                piece(g, nc.vector)
            return U

        # ---- PE warmup: bridge the pipeline-fill window (preamble + first
        # image DMA + first U transform) with dummy matmuls so the HAM
        # clock-gate lifts before the real stream starts.
        wp = psum_mm.tile([128, 128], f32, tag="ps")
        NWARM = 36
        for k in range(NWARM):
            nc.tensor.matmul(wp, lhsT=warm_w, rhs=warm_w,
                             start=(k == 0), stop=(k == NWARM - 1))

        def do_group(n, U, g, half, split_dma=False):
            """14 output rows (tile rows 7g..7g+6) of image n, one half."""
            h0 = 2 * T_GRP * g
            r = slice(T_GRP * g, T_GRP * (g + 1))
            V = [None] * 4
            # last group only: a=3 first, so its bias-fold (c3) runs during
            # the remaining matmuls and only y0/y1 trail the last one
            for a in ((3, 0, 1, 2) if split_dma else (0, 1, 2, 3)):
                ps = psum_mm.tile([128, T_GRP, W], f32, name=f"v{a}",
                                  tag="ps")
                lt = wtile[:, half, a]
                # center tap first at full width (sets has_written), then
                # the shifted taps accumulate into partial column windows
                nc.tensor.matmul(ps, lhsT=lt[:, 1], rhs=U[a][:, r, :],
                                 start=True, stop=False)
                nc.tensor.matmul(ps[:, :, 1:W], lhsT=lt[:, 0],
                                 rhs=U[a][:, r, 0:W - 1],
                                 start=False, stop=False)
                nc.tensor.matmul(ps[:, :, 0:W - 1], lhsT=lt[:, 2],
                                 rhs=U[a][:, r, 1:W],
                                 start=False, stop=True)
                V[a] = ps
            # y0 = V0+V1+V2+b, y1 = V1-V2-V3+b; ACT folds the bias into the
            # single-use terms, DVE does the two-tensor combines (max one
            # PSUM operand each).
            ob = obp.tile([128, T_GRP, 2, W], f16, name="ob", tag="ob")
            c0 = tdp.tile([128, T_GRP, W], f32, name="c0", tag="td")
            c3 = tdp.tile([128, T_GRP, W], f32, name="c3", tag="td")
            t = tdp.tile([128, T_GRP, W], f32, name="t", tag="td")
            e = tdp.tile([128, T_GRP, W], f32, name="e", tag="td")

            def act_c0():
                nc.scalar.activation(c0, V[0], AF.Identity,
                                     bias=bsb[:, half:half + 1])

            def act_c3():
                nc.scalar.activation(c3, V[3], AF.Identity,
                                     bias=bsb[:, half:half + 1], scale=-1.0)

            ych = y.ap()[n, half * 128:(half + 1) * 128]
            if split_dma:
                # tail: c3/c0/t/e all complete during the a=0..2 matmuls
                # (a=3 ran first), so only y0/y1 trail the last matmul --
                # and they go in two row-blocks so the first block's DMA
                # overlaps the second block's compute. All transfers stay
                # row-contiguous.
                act_c3(), act_c0()
                nc.vector.tensor_add(t, c0, V[1])
                nc.vector.tensor_add(e, c3, V[1])
                for rlo, rhi in ((0, 4), (4, T_GRP)):
                    nc.vector.tensor_add(ob[:, rlo:rhi, 0, :],
                                         t[:, rlo:rhi], V[2][:, rlo:rhi])
                    nc.vector.tensor_sub(ob[:, rlo:rhi, 1, :],
                                         e[:, rlo:rhi], V[2][:, rlo:rhi])
                    nc.sync.dma_start(
                        out=ych[:, h0 + 2 * rlo:h0 + 2 * rhi, :],
                        in_=ob[:, rlo:rhi],
                    )
            else:
                act_c0()
                nc.vector.tensor_add(t, c0, V[1])
                nc.vector.tensor_add(ob[:, :, 0, :], t, V[2])
                act_c3()
                nc.vector.tensor_add(e, c3, V[1])
                nc.vector.tensor_sub(ob[:, :, 1, :], e, V[2])
                nc.sync.dma_start(out=ych[:, h0:h0 + 2 * T_GRP, :], in_=ob)

        # software pipeline: staged two images ahead so GpSimd's slower
        # transform rate never gates the PE
        Us = [stage_image0(), stage_image(1), stage_image(2)]
        for n in range(N_PER_CORE):
            if n + 3 < N_PER_CORE:
                Us.append(stage_image(n + 3))
            for g in range(NGRP):
                for half in range(2):
                    last = (n == N_PER_CORE - 1 and g == NGRP - 1
                            and half == 1)
                    do_group(n, Us[n], g, half, split_dma=last)
    nc.compile()
    return nc


# F(2,3) weight transform G (exact in bf16 for +-1 weights)
_G = np.array([[1, 0, 0], [0.5, 0.5, 0.5], [0.5, -0.5, 0.5], [0, 0, 1]],
              dtype=np.float32)


def host_weight_layout(weight: np.ndarray) -> np.ndarray:
    """[256, 128, 3, 3] -> binarize, G-transform along dh,
    layout [half, i, a, dw, o] = [2, 128, 4, 3, 128] fp8-e4m3
    (values 0/+-0.5/+-1/+-1.5 are e4m3-exact)."""
    import ml_dtypes
    wc = np.clip(weight.astype(np.float32), -1.0, 1.0)
    wbin = np.where(wc >= 0, 1.0, -1.0).astype(np.float32)
    wtr = np.einsum("ad,oidw->aoiw", _G, wbin)     # [a, o, i, dw]
    w5 = wtr.reshape(4, 2, 128, C_IN, 3)           # [a, half, oo, i, dw]
    w6 = w5.transpose(1, 3, 0, 4, 2)               # [half, i, a, dw, oo]
    return np.ascontiguousarray(w6).astype(ml_dtypes.float8_e4m3fn)


def run(x, weight, bias, trace=False):
    """Returns (out [32,256,56,56] f32, BassKernelResults)."""
    import ml_dtypes
    nc = build_program()
    xb = np.asarray(x, dtype=np.float32).astype(ml_dtypes.bfloat16)
    wtr = host_weight_layout(np.asarray(weight))
    bias = np.ascontiguousarray(np.asarray(bias), dtype=np.float32)
    in_maps = [
        {
            "x": xb[i * N_PER_CORE:(i + 1) * N_PER_CORE],
            "wt": wtr,
            "b": bias,
        }
        for i in range(N_CORES)
    ]
    res = run_bass_kernel_spmd(
        nc, in_maps, core_ids=list(range(N_CORES)), trace=trace
    )
    out = np.concatenate([r["y"] for r in res.results], axis=0)
    return out.astype(np.float32), res


def kernel(x, weight, bias):
    out, _ = run(x, weight, bias)
    return out

